# revision 1
# baseline (speedup 1.0000x reference)
"""Trainium2 Bass kernel for nn_AdSBHNet (holographic Wilson-loop potential).

Strategy (pure data parallel, 8 NeuronCores, 512 batch rows/core as 4x128):
  * Host (numpy, parameter-only work): polynomial coefficients of f/df/B,
    power-series for sqrt(g), z*g'/g, g (valid on z in [0, zcap]), trapezoid
    weights (with the reference's _extend logic folded in), and the scalar
    bisection prelims (zs_max, L_max, L_crit) that depend only on a,b,logcoef.
  * Device: per-element Newton solve of L(zs)=L_eff (K steps) + the V
    integrals, all on a [batch x 1000]-point quadrature grid.
      - TensorEngine evaluates every polynomial/log-basis quantity as a
        matmul of per-row coefficient stationaries against constant y-basis
        matrices (h-1, J, sqrt(g)*y*w, z g'/g, f, g, and the z2-grid polys).
      - DVE does the data*data products, EPS guards and the fused
        multiply+reduce (scalar_tensor_tensor with accum_out) for the
        integrals.
      - ACT does ln/exp (rsqrt/sqrt via exp(+-0.5 ln x)) so the single
        natural_log_exp table set serves the whole kernel (no table thrash);
        it also builds the zs^k stationaries via exp(k*ln zs).
      - GPSIMD takes SBUF-only elementwise products off DVE's back.
Host gathers the 8 per-core [128,4] outputs and applies the validity mask.
"""
import numpy as np
from math import comb

import concourse.bass as bass
import concourse.tile as tile
from concourse import bacc, mybir
from concourse.bass_utils import run_bass_kernel_spmd
from concourse.hw_specs import get_activation_tables
import bass_rust as _bass_rust


class _PinnedActBacc(bacc.Bacc):
    """Bacc that restricts the activation-table chooser to the single
    natural_log_exp_and_others set (covers Ln/Exp/Square/Copy/Identity).
    The default chooser alternates natural_log <-> exp_and_others on every
    Ln/Exp transition, costing ~2.7us per reload (~55us/kernel)."""

    _ACT_SET = "natural_log_exp_and_others"

    def insert_act_table_loads(self):
        has_activation = any(
            isinstance(i, mybir.InstActivation)
            for b in self.main_func.blocks
            for i in b.instructions
        )
        if not has_activation:
            return
        tables = []
        for name, funcs in get_activation_tables(self.m.arch).items():
            # keep list positions (act_func_set_id is the index); empty out
            # every set except the pinned one so it is always chosen
            tables.append((name, funcs if name == self._ACT_SET else set()))
        _bass_rust.insert_act_table_loads(self, tables)

F32 = np.float32
F64 = np.float64
PI = float(np.pi)
EPS = 1e-12
B_TOTAL = 4096
N_CORES = 8
B_CORE = B_TOTAL // N_CORES      # 512
NT = 4                           # row tiles per core
P = 128                          # partitions
M = 1000                         # quadrature points
NH = 500                         # half of M (PSUM bank granularity)
KZ = 64                          # zs-power series order
NEWTON_STEPS = 1
DT = mybir.dt.float32

_CACHE = {}


# ----------------------------------------------------------------------------
# Host-side math (parameter-only, O(M) work)
# ----------------------------------------------------------------------------

def _ygrid():
    return np.linspace(0.001, 0.999, M, dtype=F32).astype(F64)


def _trapz_weights():
    y = _ygrid()
    y0 = y[0]
    h = (y[-1] - y[0]) / (M - 1)
    w = np.full(M, h, F64)
    w[0] = 0.5 * h + y0 + 0.5 * y0 * y0 / h
    w[1] = h - 0.5 * y0 * y0 / h
    w[-1] = 0.5 * h + 0.5 * (1.0 - y[-1])
    return w


def _y2grid():
    return np.linspace(0.001, 1.0, M, dtype=F32).astype(F64)


def _trapz2_weights():
    y2 = _y2grid()
    h2 = (y2[-1] - y2[0]) / (M - 1)
    w2 = np.full(M, h2, F64)
    w2[0] = 0.5 * h2 + 0.5 * y2[0]
    w2[-1] = 0.5 * h2
    return w2, 0.5 * y2[0]


def _f_coeffs(a):
    _a = np.concatenate([np.ones(1, F64), np.asarray(a, F64)])
    A = np.zeros(5, F64)
    q = 0.0
    for i in range(3):
        for j in range(3):
            cc = _a[i] * _a[j]
            if i + j == 4:
                q += -4.0 * cc
            else:
                A[4] += 4.0 * cc / (i + j - 4)
                A[i + j] -= 4.0 * cc / (i + j - 4)
    return A, q


def _df_coeffs(a):
    _a = np.concatenate([np.ones(1, F64), np.asarray(a, F64)])
    A, q = _f_coeffs(a)
    D = 4.0 * A.copy()
    for i in range(3):
        for j in range(3):
            D[i + j] -= 4.0 * _a[i] * _a[j]
    return D, 4.0 * q


def _b_coeffs(a, b):
    last = float(np.asarray(a, F64).sum()) - float(np.asarray(b, F64).sum())
    return np.array([1.0, float(b[0]), float(b[1]), last], F64)


def _series_inv_poly(c, K):
    e = np.zeros(K)
    e[0] = 1.0 / c[0]
    for k in range(1, K):
        s = 0.0
        for j in range(1, min(len(c), k + 1)):
            s += c[j] * e[k - j]
        e[k] = -s / c[0]
    return e


def _conv_trunc(a, b, K):
    return np.convolve(a, b)[:K]


def _build_series(c):
    n_terms = KZ // 4 + 1
    s = np.zeros(n_terms)
    s[0] = 1.0
    for n in range(1, n_terms):
        s[n] = s[n - 1] * (2 * n - 1) / (2 * n)
    rsq = np.zeros(KZ)
    rsq[::4] = s[: len(rsq[::4])]
    sg = _conv_trunc(c, rsq, KZ)                       # B(z)(1-z^4)^-1/2
    zBp = np.array([0.0, c[1], 2 * c[2], 3 * c[3]])
    g1 = 2.0 * _conv_trunc(zBp, _series_inv_poly(c, KZ), KZ)
    g2 = np.zeros(KZ)
    g2[4::4] = 4.0
    gg = g1 + g2                                       # z g'/g
    inv1mz4 = np.zeros(KZ)
    inv1mz4[::4] = 1.0
    gser = _conv_trunc(_conv_trunc(c, c, KZ), inv1mz4, KZ)  # B^2/(1-z^4)
    return sg, gg, gser


class _HostModel:
    """float32 replica of the reference for the scalar bisection prelims."""

    def __init__(self, a, b):
        self.A, self.q = _f_coeffs(a)
        self.D, self.dq = _df_coeffs(a)
        self.c = _b_coeffs(a, b)
        self.y = _ygrid().astype(F32)
        self.u = ((1 - self.y) * (1 + self.y)).astype(F32)
        self.w = _trapz_weights().astype(F32)
        self.y2 = _y2grid().astype(F32)
        w2, c2 = _trapz2_weights()
        self.w2 = w2.astype(F32)
        self.c2 = F32(c2)

    def _f(self, z, lnz):
        A, q = self.A, self.q
        return (A[4] * z**4 + A[3] * z**3 + A[2] * z**2 + A[1] * z + A[0]
                + q * z**4 * lnz).astype(F32)

    def _df(self, z, lnz):
        D, dq = self.D, self.dq
        return (D[0] / z + D[1] + D[2] * z + D[3] * z**2 + D[4] * z**3
                + dq * z**3 * lnz).astype(F32)

    def L_dL(self, zs):
        zs = np.asarray(zs, F32).reshape(-1)[:, None]
        u, y, w = self.u[None, :], self.y[None, :], self.w
        z = (zs * u).astype(F32)
        lnz = np.log(z)
        lnzs = np.log(zs)
        fs = self._f(zs, lnzs)
        dfs = self._df(zs, lnzs)
        rfs = (1.0 / fs).astype(F32)
        f = self._f(z, lnz)
        c = self.c
        Bv = (c[0] + c[1] * z + c[2] * z**2 + c[3] * z**3).astype(F32)
        Bp = (c[1] + 2 * c[2] * z + 3 * c[3] * z**2).astype(F32)
        D_ = (1 - z**4).astype(F32)
        sqrtg = (Bv / np.sqrt(D_)).astype(F32)
        h = (f * rfs / u**4).astype(F32)
        m = np.maximum(h - 1, F32(EPS))
        R = (1.0 / np.sqrt(m)).astype(F32)
        TL = ((sqrtg * R * y * w).sum(-1, dtype=F64)).astype(F32)
        L = (4.0 * zs[:, 0] * TL / PI).astype(F32)
        G = (2 * z * Bp / Bv + 4 * z**4 / D_).astype(F32)
        sA = (zs * dfs * rfs + 2).astype(F32)
        J = (zs**4 / z**3 * self._df(z, lnz) * rfs).astype(F32)
        v = (h * (sA + G) - J - 2 - G).astype(F32)
        IdL = (v * 2 * y * sqrtg * R / m).astype(F32)
        dL = ((IdL * w).sum(-1, dtype=F64) / PI).astype(F32)
        return L, dL

    def V(self, zs, coef):
        zs = np.asarray(zs, F32).reshape(-1)[:, None]
        u, y, w = self.u[None, :], self.y[None, :], self.w
        z = (zs * u).astype(F32)
        lnz = np.log(z)
        lnzs = np.log(zs)
        fs = self._f(zs, lnzs)
        f = self._f(z, lnz)
        c = self.c
        Bv = (c[0] + c[1] * z + c[2] * z**2 + c[3] * z**3).astype(F32)
        g = (Bv * Bv / (1 - z**4)).astype(F32)
        fg = np.maximum(f * g, F32(EPS))
        arg = np.maximum(1 - u**4 * fs / f, F32(EPS))
        integ = (np.sqrt(fg) / u**2 * (1 / np.sqrt(arg) - 1) * y).astype(F32)
        Vc = (coef * PI * 4.0 * (integ * w).sum(-1, dtype=F64) / zs[:, 0]).astype(F32)
        y2, w2 = self.y2[None, :], self.w2
        z2 = (1 - (1 - zs) * y2).astype(F32)
        f2 = self._f(z2, np.log(z2))
        B2 = (c[0] + c[1] * z2 + c[2] * z2**2 + c[3] * z2**3).astype(F32)
        g2 = (B2 * B2 / (1 - z2**4)).astype(F32)
        fg2 = np.maximum(f2 * g2, F32(EPS))
        integ2 = (np.sqrt(fg2) / z2**2).astype(F32)
        Vd = (coef * PI * 2.0 * (1 - zs[:, 0])
              * ((integ2 * w2).sum(-1, dtype=F64) + self.c2)).astype(F32)
        return (Vc - Vd).astype(F32)

    def bisect(self, fun, lo, hi, iters=30):
        lo, hi = F32(lo), F32(hi)
        for _ in range(iters):
            mid = F32(0.5) * (lo + hi)
            if fun(mid) < 0:
                hi = mid
            else:
                lo = mid
        return F32(0.5) * (lo + hi)

    def prelims(self, coef):
        zs_max = self.bisect(lambda mm: self.L_dL(mm)[1][0], 0.001, 0.999)
        L_max = self.L_dL(zs_max)[0][0]
        zs_crit = self.bisect(lambda mm: -self.V(mm, coef)[0], 0.001, zs_max)
        L_crit = self.L_dL(zs_crit)[0][0]
        return zs_max, L_max, zs_crit, L_crit


def _host_build(a, b, logcoef):
    """All parameter-derived constants + basis matrices."""
    A, q = _f_coeffs(a)
    Dc, dq = _df_coeffs(a)
    c = _b_coeffs(a, b)
    sg, gg, gser = _build_series(c)
    coef = float(np.exp(F32(np.asarray(logcoef).reshape(-1)[0]
                            if np.ndim(logcoef) else logcoef)))

    mdl = _HostModel(a, b)
    zs_max, L_max, zs_crit, L_crit = mdl.prelims(coef)
    zcap = float(min(0.9995, float(zs_max) * 0.97))
    zgrid = np.linspace(1e-4, zcap, 257).astype(F32)
    Lgrid = mdl.L_dL(zgrid)[0]

    y = _ygrid()
    u = 1 - y * y
    lnu = np.log(u)
    w = _trapz_weights()
    yw = y * w
    ru = 1.0 / u
    ks = np.arange(KZ)[:, None]

    BH = np.stack([A[0] * ru**4, A[1] * ru**3, A[2] * ru**2, A[3] * ru,
                   A[4] + q * lnu, q * np.ones(M), -np.ones(M),
                   np.zeros(M)]).astype(F32)
    BJ = np.stack([Dc[0] * ru**4, Dc[1] * ru**3, Dc[2] * ru**2, Dc[3] * ru,
                   Dc[4] + dq * lnu, dq * np.ones(M), np.zeros(M),
                   np.ones(M)]).astype(F32)
    BSGW = (sg[:, None] * u[None, :]**ks * yw[None, :]).astype(F32)
    BG = (gg[:, None] * u[None, :]**ks).astype(F32)

    c2w = (yw / u**2)**2
    BFV = np.stack([A[0] * c2w, A[1] * u * c2w, A[2] * u**2 * c2w,
                    A[3] * u**3 * c2w, (A[4] + q * lnu) * u**4 * c2w,
                    q * u**4 * c2w]).astype(F32)
    BGV = (gser[:, None] * u[None, :]**ks).astype(F32)
    U4C2 = (u**4 * (yw / u**2)**2).astype(F32)

    y2 = _y2grid()
    w2, c2 = _trapz2_weights()
    alpha, beta = 1 - y2, y2
    w2s = w2 * w2

    def phi(coefs, extra, mmax):
        rows = []
        for mdeg in range(mmax):
            r = np.zeros(M)
            for k in range(mdeg, len(coefs)):
                if coefs[k] != 0:
                    r += coefs[k] * comb(k, mdeg) * alpha**(k - mdeg) * beta**mdeg
            rows.append(r * extra)
        return np.stack(rows).astype(F32)

    PHI_Z2 = phi([0, 1], np.ones(M), 2)
    PHI_FD = phi(list(A), w2s, 5)
    PHI_B = phi(list(c), np.ones(M), 4)
    PHI_D2 = phi([1, 0, 0, 0, -1], np.ones(M), 5)
    PHI_Z4 = phi([0, 0, 0, 0, 1], w2s, 5)

    return dict(
        A=A, q=q, Dc=Dc, dq=dq, c=c, coef=coef, c2=float(c2),
        zs_max=float(zs_max), L_max=float(L_max), L_crit=float(L_crit),
        zcap=zcap, zgrid=zgrid, Lgrid=Lgrid,
        BH=BH, BJ=BJ, BSGW=BSGW, BG=BG, BFV=BFV, BGV=BGV, U4C2=U4C2,
        PHI_Z2=PHI_Z2, PHI_FD=PHI_FD, PHI_B=PHI_B, PHI_D2=PHI_D2,
        PHI_Z4=PHI_Z4,
    )


# ----------------------------------------------------------------------------
# Device graph
# ----------------------------------------------------------------------------

def _build_graph(host):
    A, q, Dc, dq = host["A"], host["q"], host["Dc"], host["dq"]
    coef, c2 = host["coef"], host["c2"]
    zcap = host["zcap"]
    f32 = lambda x: float(F32(x))
    alu = mybir.AluOpType
    act = mybir.ActivationFunctionType

    nc = _PinnedActBacc("TRN2", target_bir_lowering=False, debug=False,
                        num_devices=N_CORES)

    def param(name, shape, dt_=DT):
        return nc.declare_dram_parameter(name, list(shape), dt_, isOutput=False)

    lt_ext = param("lt", [P, NT])
    init_ext = param("init", [P, NT])
    bh_ext = param("bh", [8, M], DT)
    bj_ext = param("bj", [8, M], DT)
    bsgw_ext = param("bsgw", [KZ, M], DT)
    bg_ext = param("bg", [KZ, M], DT)
    bfv_ext = param("bfv", [6, M], DT)
    bgv_ext = param("bgv", [KZ, M], mybir.dt.bfloat16)
    phiz2_ext = param("phiz2", [2, M], DT)
    phifd_ext = param("phifd", [5, M], mybir.dt.bfloat16)
    phib_ext = param("phib", [4, M], mybir.dt.bfloat16)
    phid2_ext = param("phid2", [5, M], mybir.dt.bfloat16)
    phiz4_ext = param("phiz4", [5, M], mybir.dt.bfloat16)
    u4c2_ext = param("u4c2", [1, M])
    iota_ext = param("iota", [1, KZ])
    ident_ext = param("ident", [P, P])
    out_ext = nc.declare_dram_parameter("out", [P, NT], DT, isOutput=True)
    zs_ext = nc.declare_dram_parameter("zsdbg", [P, NT], DT, isOutput=True)

    with tile.TileContext(nc) as tc:
        with (
            tc.tile_pool(name="const", bufs=1) as cpool,
            tc.tile_pool(name="state", bufs=2) as spool,
            tc.tile_pool(name="sc", bufs=1) as scpool,
            tc.tile_pool(name="small", bufs=3) as smpool,
            tc.tile_pool(name="stat", bufs=3) as stpool,
            tc.tile_pool(name="big", bufs=2) as bpool,
            tc.tile_pool(name="big3", bufs=3) as b3pool,
            tc.tile_pool(name="vbig", bufs=1) as vpool,
            tc.tile_pool(name="vbig2", bufs=2) as v2pool,
            tc.tile_pool(name="psum", bufs=2, space="PSUM") as ppool,
            tc.tile_pool(name="psum1", bufs=1, space="PSUM") as p1pool,
        ):
            # ---- constants to SBUF ----
            def cload(ext, shape, tag, dt_=DT):
                t = cpool.tile(list(shape), dt_, tag=tag)
                nc.sync.dma_start(t[:], ext[:])
                return t

            # order matters: HWDGE serves DMAs roughly in issue order, so
            # the tiny inputs + Newton-critical bases must precede the
            # V-only constants or the whole pipeline stalls ~10us at start
            LTT = cload(lt_ext, [P, NT], "c_lt")
            IOTA = cload(iota_ext, [1, KZ], "c_iota")
            IDENT = cload(ident_ext, [P, P], "c_ident")
            BH = cload(bh_ext, [8, M], "c_bh", DT)
            BJ = cload(bj_ext, [8, M], "c_bj", DT)
            BSGW = cload(bsgw_ext, [KZ, M], "c_bsgw", DT)
            BG = cload(bg_ext, [KZ, M], "c_bg", DT)
            BFV = cload(bfv_ext, [6, M], "c_bfv", DT)
            BGV = cload(bgv_ext, [KZ, M], "c_bgv", mybir.dt.bfloat16)
            PHIZ2 = cload(phiz2_ext, [2, M], "c_phiz2", DT)
            PHIFD = cload(phifd_ext, [5, M], "c_phifd", mybir.dt.bfloat16)
            PHIB = cload(phib_ext, [4, M], "c_phib", mybir.dt.bfloat16)
            PHID2 = cload(phid2_ext, [5, M], "c_phid2", mybir.dt.bfloat16)
            PHIZ4 = cload(phiz4_ext, [5, M], "c_phiz4", mybir.dt.bfloat16)

            # ---- state (issued early; see DMA-order note above) ----
            ZS = spool.tile([P, NT], DT, tag="zs")
            nc.sync.dma_start(ZS[:], init_ext[:])
            LNZS = spool.tile([P, NT], DT, tag="lnzs")
            nc.scalar.activation(LNZS[:], ZS[:], act.Ln)

            U4C2R = cpool.tile([1, M], DT, tag="c_u4c2r")
            nc.sync.dma_start(U4C2R[:], u4c2_ext[:])
            U4C2 = cpool.tile([P, M], DT, tag="c_u4c2")
            nc.gpsimd.partition_broadcast(U4C2[:], U4C2R[:])

            # persistent scalar-coef assembly tiles (tile-major layout)
            SC = scpool.tile([P, NT * 8], DT, tag="sc")    # col = t*8+q
            SCV = scpool.tile([P, NT * 6], DT, tag="scv")  # col = t*6+q
            nc.vector.memset(SC[:].rearrange("p (t q) -> p t q", q=8)[:, :, 6], 1.0)
            nc.vector.memset(SCV[:].rearrange("p (t q) -> p t q", q=6)[:, :, 0], 1.0)

            def mm(out_ap, lhsT_ap, rhs_ap):
                nc.tensor.matmul(out_ap, lhsT_ap, rhs_ap)

            def sc_q(tile_, nq, qidx):
                return tile_[:].rearrange("p (t q) -> p t q", q=nq)[:, :, qidx]

            def sc_t(tile_, nq, t):
                return tile_[:].rearrange("p (t q) -> p t q", q=nq)[:, t, :]

            def build_powers_stationaries(nrows, with_bf16=False):
                """lnzs row-transpose + per-tile zs^k stationaries via exp."""
                S2s = []
                for t in range(NT):
                    LTp = p1pool.tile([1, P], DT, tag="tp")
                    nc.tensor.transpose(LTp[:], LNZS[:, t:t + 1], IDENT[:])
                    LTs = smpool.tile([1, P], DT, tag=f"ltrow_{t}")
                    nc.scalar.copy(LTs[:], LTp[:])
                    KLN = p1pool.tile([nrows, P], DT, tag="kln")
                    nc.tensor.matmul(KLN[:], IOTA[:1, :nrows], LTs[0:1, :])
                    S2t = stpool.tile([nrows, P], DT, tag=f"s2_{t}")
                    nc.scalar.activation(S2t[:], KLN[:], act.Exp)
                    if with_bf16:
                        S2b = stpool.tile([nrows, P], mybir.dt.bfloat16,
                                          tag=f"s2b_{t}")
                        nc.scalar.activation(S2b[:], KLN[:], act.Exp)
                    else:
                        S2b = None
                    S2s.append((S2t, S2b))
                return S2s

            def transpose_sc(src_ap, rows, t, with_bf16=False):
                TPp = p1pool.tile([rows, P], DT, tag="tp")
                nc.tensor.transpose(TPp[:], src_ap, IDENT[:])
                S1t = stpool.tile([rows, P], DT, tag=f"s1_{t}")
                nc.scalar.copy(S1t[:], TPp[:])
                S1b = None
                if with_bf16:
                    S1b = stpool.tile([rows, P], mybir.dt.bfloat16,
                                      tag=f"s1b_{t}")
                    nc.scalar.copy(S1b[:], TPp[:])
                return S1t, S1b

            # ================= Newton iterations =================
            for step in range(NEWTON_STEPS):
                # ---- per-row scalar phase ([P, NT]) ----
                ZS2 = smpool.tile([P, NT], DT, tag="zs2")
                nc.vector.tensor_mul(ZS2[:], ZS[:], ZS[:])
                ZS3 = smpool.tile([P, NT], DT, tag="zs3")
                nc.vector.tensor_mul(ZS3[:], ZS2[:], ZS[:])
                ZS4 = smpool.tile([P, NT], DT, tag="zs4")
                nc.vector.tensor_mul(ZS4[:], ZS2[:], ZS2[:])
                RZS = smpool.tile([P, NT], DT, tag="rzs")
                nc.vector.reciprocal(RZS[:], ZS[:])
                LZ4 = smpool.tile([P, NT], DT, tag="lz4")
                nc.vector.tensor_mul(LZ4[:], ZS4[:], LNZS[:])
                LZ3 = smpool.tile([P, NT], DT, tag="lz3")
                nc.vector.tensor_mul(LZ3[:], ZS3[:], LNZS[:])

                FS = smpool.tile([P, NT], DT, tag="fs")
                t1 = smpool.tile([P, NT], DT, tag="tmp1")
                nc.vector.tensor_scalar(t1[:], ZS[:], f32(A[1]), f32(A[0]),
                                        alu.mult, alu.add)
                t2 = smpool.tile([P, NT], DT, tag="tmp2")
                nc.vector.scalar_tensor_tensor(t2[:], ZS2[:], f32(A[2]), t1[:],
                                               alu.mult, alu.add)
                nc.vector.scalar_tensor_tensor(t1[:], ZS3[:], f32(A[3]), t2[:],
                                               alu.mult, alu.add)
                nc.vector.scalar_tensor_tensor(t2[:], ZS4[:], f32(A[4]), t1[:],
                                               alu.mult, alu.add)
                nc.vector.scalar_tensor_tensor(FS[:], LZ4[:], f32(q), t2[:],
                                               alu.mult, alu.add)

                DFS = smpool.tile([P, NT], DT, tag="dfs")
                t3 = smpool.tile([P, NT], DT, tag="tmp3")
                nc.vector.tensor_scalar(t3[:], ZS[:], f32(Dc[2]), f32(Dc[1]),
                                        alu.mult, alu.add)
                t4 = smpool.tile([P, NT], DT, tag="tmp4")
                nc.vector.scalar_tensor_tensor(t4[:], ZS2[:], f32(Dc[3]), t3[:],
                                               alu.mult, alu.add)
                nc.vector.scalar_tensor_tensor(t3[:], ZS3[:], f32(Dc[4]), t4[:],
                                               alu.mult, alu.add)
                nc.vector.scalar_tensor_tensor(t4[:], RZS[:], f32(Dc[0]), t3[:],
                                               alu.mult, alu.add)
                nc.vector.scalar_tensor_tensor(DFS[:], LZ3[:], f32(dq), t4[:],
                                               alu.mult, alu.add)

                RFS = smpool.tile([P, NT], DT, tag="rfs")
                nc.vector.reciprocal(RFS[:], FS[:])
                T0 = smpool.tile([P, NT], DT, tag="t0")
                nc.gpsimd.tensor_mul(T0[:], ZS[:], DFS[:])
                M2SA = smpool.tile([P, NT], DT, tag="m2sa")
                nc.vector.scalar_tensor_tensor(M2SA[:], T0[:], -1.0, RFS[:],
                                               alu.mult, alu.mult)
                SA = smpool.tile([P, NT], DT, tag="sa")
                nc.vector.tensor_scalar(SA[:], M2SA[:], -1.0, 2.0,
                                        alu.mult, alu.add)

                # SC assembly: cols t*8+q; q = rfs, rfs zs^1..4, rfs zs4 lnzs,
                # 1 (static), 2-sA
                nc.gpsimd.tensor_copy(sc_q(SC, 8, 0), RFS[:])
                nc.vector.tensor_mul(sc_q(SC, 8, 1), RFS[:], ZS[:])
                nc.vector.tensor_mul(sc_q(SC, 8, 2), RFS[:], ZS2[:])
                nc.vector.tensor_mul(sc_q(SC, 8, 3), RFS[:], ZS3[:])
                nc.vector.tensor_mul(sc_q(SC, 8, 4), RFS[:], ZS4[:])
                nc.vector.tensor_mul(sc_q(SC, 8, 5), sc_q(SC, 8, 4), LNZS[:])
                nc.gpsimd.tensor_copy(sc_q(SC, 8, 7), M2SA[:])

                S2s = build_powers_stationaries(KZ)

                TLA = smpool.tile([P, NT], DT, tag="tla")
                TLB = smpool.tile([P, NT], DT, tag="tlb")
                TD1A = smpool.tile([P, NT], DT, tag="tda")
                TD1B = smpool.tile([P, NT], DT, tag="tdb")
                TD2A = smpool.tile([P, NT], DT, tag="td2a")
                TD2B = smpool.tile([P, NT], DT, tag="td2b")

                for t in range(NT):
                    S1t, _ = transpose_sc(sc_t(SC, 8, t), 8, t)
                    # TD = sum (G+sA)*SW - sum Jp*rm*SW   (uses H1*rm == 1)
                    for h in range(2):
                        sl = slice(h * NH, (h + 1) * NH)
                        HJ = ppool.tile([P, 1024], DT, tag="hj")
                        H1 = HJ[:, 0:NH]
                        mm(H1, S1t[:], BH[:, sl])
                        JP = HJ[:, 512:512 + NH]
                        mm(JP, S1t[:], BJ[:, sl])
                        SG_ = p1pool.tile([P, 1024], DT, tag="sg")
                        SGW = SG_[:, 0:NH]
                        mm(SGW, S2s[t][0][:], BSGW[:, sl])
                        G = SG_[:, 512:512 + NH]
                        mm(G, S2s[t][0][:], BG[:, sl])

                        GsA = bpool.tile([P, NH], DT, tag="mx")
                        nc.scalar.activation(GsA[:], G, act.Identity,
                                             bias=SA[:, t:t + 1])
                        SGs = bpool.tile([P, NH], DT, tag="scr")
                        nc.scalar.copy(SGs[:], SGW)
                        LM = b3pool.tile([P, NH], DT, tag="lm")
                        nc.scalar.activation(LM[:], H1, act.Ln)
                        R = b3pool.tile([P, NH], DT, tag="w1")
                        nc.scalar.activation(R[:], LM[:], act.Exp, scale=-0.5)
                        RM = b3pool.tile([P, NH], DT, tag="rm")
                        nc.gpsimd.tensor_mul(RM[:], R[:], R[:])
                        SW = b3pool.tile([P, NH], DT, tag="sw")
                        nc.vector.scalar_tensor_tensor(
                            SW[:], SGs[:], 1.0, R[:],
                            alu.mult, alu.mult,
                            accum_out=(TLA[:, t:t + 1] if h == 0
                                       else TLB[:, t:t + 1]))
                        RSW = bpool.tile([P, NH], DT, tag="d1")
                        nc.gpsimd.tensor_mul(RSW[:], RM[:], SW[:])
                        SC1 = bpool.tile([P, NH], DT, tag="tlm")
                        nc.vector.scalar_tensor_tensor(
                            SC1[:], GsA[:], 1.0, SW[:], alu.mult, alu.mult,
                            accum_out=(TD1A[:, t:t + 1] if h == 0
                                       else TD1B[:, t:t + 1]))
                        SC2 = bpool.tile([P, NH], DT, tag="fd2")
                        nc.vector.scalar_tensor_tensor(
                            SC2[:], JP, 1.0, RSW[:], alu.mult, alu.mult,
                            accum_out=(TD2A[:, t:t + 1] if h == 0
                                       else TD2B[:, t:t + 1]))

                # ---- Newton update ----
                TLS = smpool.tile([P, NT], DT, tag="tls")
                nc.vector.tensor_add(TLS[:], TLA[:], TLB[:])
                TD1S = smpool.tile([P, NT], DT, tag="td1s")
                nc.vector.tensor_add(TD1S[:], TD1A[:], TD1B[:])
                TD2S = smpool.tile([P, NT], DT, tag="td2s")
                nc.vector.tensor_add(TD2S[:], TD2A[:], TD2B[:])
                TDS = smpool.tile([P, NT], DT, tag="tds")
                nc.vector.tensor_sub(TDS[:], TD1S[:], TD2S[:])
                T1f = smpool.tile([P, NT], DT, tag="t1f")
                nc.vector.tensor_mul(T1f[:], ZS[:], TLS[:])
                RT = smpool.tile([P, NT], DT, tag="rt")
                nc.vector.reciprocal(RT[:], TDS[:])
                LMF = smpool.tile([P, NT], DT, tag="lmf")
                nc.vector.scalar_tensor_tensor(LMF[:], T1f[:], f32(4.0 / PI),
                                               LTT[:], alu.mult, alu.subtract)
                DEL = smpool.tile([P, NT], DT, tag="del")
                nc.vector.scalar_tensor_tensor(DEL[:], LMF[:], f32(PI / 2),
                                               RT[:], alu.mult, alu.mult)
                ZSn = smpool.tile([P, NT], DT, tag="zsn")
                nc.vector.tensor_sub(ZSn[:], ZS[:], DEL[:])
                ZS = spool.tile([P, NT], DT, tag="zs")
                nc.vector.tensor_scalar(ZS[:], ZSn[:], 1e-4, zcap,
                                        alu.max, alu.min)
                LNZS = spool.tile([P, NT], DT, tag="lnzs")
                nc.scalar.activation(LNZS[:], ZS[:], act.Ln)

            # ================= V phase =================
            ZS2 = smpool.tile([P, NT], DT, tag="zs2")
            nc.vector.tensor_mul(ZS2[:], ZS[:], ZS[:])
            ZS3 = smpool.tile([P, NT], DT, tag="zs3")
            nc.vector.tensor_mul(ZS3[:], ZS2[:], ZS[:])
            ZS4 = smpool.tile([P, NT], DT, tag="zs4")
            nc.vector.tensor_mul(ZS4[:], ZS2[:], ZS2[:])
            LZ4 = smpool.tile([P, NT], DT, tag="lz4")
            nc.vector.tensor_mul(LZ4[:], ZS4[:], LNZS[:])
            FS = smpool.tile([P, NT], DT, tag="fs")
            t1 = smpool.tile([P, NT], DT, tag="tmp1")
            nc.vector.tensor_scalar(t1[:], ZS[:], f32(A[1]), f32(A[0]),
                                    alu.mult, alu.add)
            t2 = smpool.tile([P, NT], DT, tag="tmp2")
            nc.vector.scalar_tensor_tensor(t2[:], ZS2[:], f32(A[2]), t1[:],
                                           alu.mult, alu.add)
            nc.vector.scalar_tensor_tensor(t1[:], ZS3[:], f32(A[3]), t2[:],
                                           alu.mult, alu.add)
            nc.vector.scalar_tensor_tensor(t2[:], ZS4[:], f32(A[4]), t1[:],
                                           alu.mult, alu.add)
            nc.vector.scalar_tensor_tensor(FS[:], LZ4[:], f32(q), t2[:],
                                           alu.mult, alu.add)
            RZSv = smpool.tile([P, NT], DT, tag="rzsv")
            nc.vector.reciprocal(RZSv[:], ZS[:])

            # SCV assembly: q = {1 (static), zs, zs2, zs3, zs4, zs4 lnzs}
            nc.gpsimd.tensor_copy(sc_q(SCV, 6, 1), ZS[:])
            nc.gpsimd.tensor_copy(sc_q(SCV, 6, 2), ZS2[:])
            nc.gpsimd.tensor_copy(sc_q(SCV, 6, 3), ZS3[:])
            nc.gpsimd.tensor_copy(sc_q(SCV, 6, 4), ZS4[:])
            nc.gpsimd.tensor_copy(sc_q(SCV, 6, 5), LZ4[:])

            S2s = build_powers_stationaries(KZ, with_bf16=True)

            T2A = smpool.tile([P, NT], DT, tag="t2a")
            T2B = smpool.tile([P, NT], DT, tag="t2b")
            T1A = smpool.tile([P, NT], DT, tag="t1a")
            T1B = smpool.tile([P, NT], DT, tag="t1b")
            TDA = smpool.tile([P, NT], DT, tag="tda")
            TDB = smpool.tile([P, NT], DT, tag="tdb")

            for t in range(NT):
                SVt, SVb = transpose_sc(sc_t(SCV, 6, t), 6, t, with_bf16=True)
                FG = v2pool.tile([P, M], DT, tag="vfg")
                IFV = v2pool.tile([P, M], DT, tag="vifv")
                LZ2 = v2pool.tile([P, M], DT, tag="vlz2")
                ID2 = v2pool.tile([P, M], DT, tag="vid2")
                B2S = v2pool.tile([P, M], DT, tag="vb2s")
                FD2 = v2pool.tile([P, M], DT, tag="vfd2")
                for h in range(2):
                    sl = slice(h * NH, (h + 1) * NH)
                    # connected matmuls + PSUM consumers (500-wide)
                    FG_ = ppool.tile([P, 1024], DT, tag="hj")
                    FV = FG_[:, 0:NH]
                    mm(FV, SVt[:], BFV[:, sl])
                    GV = FG_[:, 512:512 + NH]
                    mm(GV, S2s[t][1][:], BGV[:, sl])
                    GVs = bpool.tile([P, NH], DT, tag="lm")
                    nc.scalar.copy(GVs[:], GV)
                    nc.vector.tensor_mul(FG[:, sl], FV, GVs[:])
                    nc.vector.reciprocal_approx_fast(IFV[:, sl], FV)
                    # disconnected matmuls + PSUM consumers (500-wide)
                    ZF_ = ppool.tile([P, 1024], DT, tag="hj")
                    Z2 = ZF_[:, 0:NH]
                    mm(Z2, SVt[0:2, :], PHIZ2[:, sl])
                    FDW = ZF_[:, 512:512 + NH]
                    mm(FDW, SVb[0:5, :], PHIFD[:, sl])
                    BD_ = p1pool.tile([P, 1024], DT, tag="sg")
                    B2D = BD_[:, 0:NH]
                    mm(B2D, SVb[0:4, :], PHIB[:, sl])
                    D2 = BD_[:, 512:512 + NH]
                    mm(D2, SVb[0:5, :], PHID2[:, sl])
                    nc.scalar.activation(LZ2[:, sl], Z2, act.Ln)
                    nc.vector.reciprocal_approx_fast(ID2[:, sl], D2)
                    nc.scalar.activation(B2S[:, sl], B2D, act.Square)
                    Z4W = p1pool.tile([P, NH], DT, tag="kln")
                    mm(Z4W[:], SVb[0:5, :], PHIZ4[:, sl])
                    TLm = bpool.tile([P, NH], DT, tag="tlm")
                    nc.vector.tensor_mul(TLm[:], Z4W[:], LZ2[:, sl])
                    nc.vector.scalar_tensor_tensor(FD2[:, sl], TLm[:], f32(q),
                                                   FDW, alu.mult, alu.add)
                # full-width tail (1000-wide)
                LFG = vpool.tile([P, M], DT, tag="vlfg")
                nc.scalar.activation(LFG[:], FG[:], act.Ln)
                SQA = vpool.tile([P, M], DT, tag="vsqa")
                nc.scalar.activation(SQA[:], LFG[:], act.Exp, scale=0.5,
                                     accum_out=T2A[:, t:t + 1])
                T2t = vpool.tile([P, M], DT, tag="vt2t")
                nc.vector.scalar_tensor_tensor(
                    T2t[:], IFV[:], FS[:, t:t + 1], U4C2[:],
                    alu.mult, alu.mult)
                LAR = vpool.tile([P, M], DT, tag="vlar")
                nc.scalar.activation(LAR[:], T2t[:], act.Ln,
                                     bias=1.0, scale=-1.0)
                LCOC = vpool.tile([P, M], DT, tag="vlcoc")
                nc.vector.scalar_tensor_tensor(
                    LCOC[:], LAR[:], -1.0, LFG[:], alu.mult, alu.add)
                EXPC = vpool.tile([P, M], DT, tag="vexpc")
                nc.scalar.activation(EXPC[:], LCOC[:], act.Exp, scale=0.5,
                                     accum_out=T1A[:, t:t + 1])
                FG2a = vpool.tile([P, M], DT, tag="vfg2a")
                nc.gpsimd.tensor_mul(FG2a[:], FD2[:], ID2[:])
                FG2 = vpool.tile([P, M], DT, tag="vfg2")
                nc.gpsimd.tensor_mul(FG2[:], FG2a[:], B2S[:])
                LF2 = vpool.tile([P, M], DT, tag="vlf2")
                nc.scalar.activation(LF2[:], FG2[:], act.Ln)
                LCOD = vpool.tile([P, M], DT, tag="vlcod")
                nc.vector.scalar_tensor_tensor(
                    LCOD[:], LZ2[:], -4.0, LF2[:], alu.mult, alu.add)
                EXPD = vpool.tile([P, M], DT, tag="vexpd")
                nc.scalar.activation(EXPD[:], LCOD[:], act.Exp, scale=0.5,
                                     accum_out=TDA[:, t:t + 1])

            # ---- V finalize ----
            TVS = TDA
            TVC = smpool.tile([P, NT], DT, tag="tvc")
            nc.vector.tensor_sub(TVC[:], T1A[:], T2A[:])
            VC1 = smpool.tile([P, NT], DT, tag="vc1")
            nc.vector.tensor_mul(VC1[:], TVC[:], RZSv[:])
            O1 = smpool.tile([P, NT], DT, tag="o1")
            nc.vector.tensor_scalar(O1[:], VC1[:], f32(4.0 * PI * coef), None,
                                    alu.mult)
            TVDc = smpool.tile([P, NT], DT, tag="tvdc")
            nc.vector.tensor_scalar(TVDc[:], TVS[:], f32(c2), None, alu.add)
            OMZ = smpool.tile([P, NT], DT, tag="omz")
            nc.vector.tensor_scalar(OMZ[:], ZS[:], -1.0, 1.0, alu.mult,
                                    alu.add)
            VD1 = smpool.tile([P, NT], DT, tag="vd1")
            nc.vector.tensor_mul(VD1[:], TVDc[:], OMZ[:])
            OUT = smpool.tile([P, NT], DT, tag="outt")
            nc.vector.scalar_tensor_tensor(OUT[:], VD1[:],
                                           f32(-2.0 * PI * coef), O1[:],
                                           alu.mult, alu.add)
            nc.sync.dma_start(out_ext[:], OUT[:])
            nc.sync.dma_start(zs_ext[:], ZS[:])

    nc.compile()
    return nc


# ----------------------------------------------------------------------------
# kernel entry point
# ----------------------------------------------------------------------------

def kernel(Ls, a, b, logcoef):
    Ls_in = np.asarray(Ls, F32).reshape(-1)
    n_in = Ls_in.size
    if n_in == B_TOTAL:
        Ls = Ls_in
    else:
        # harness contract is B=4096; pad/trim defensively
        Ls = np.full(B_TOTAL, 0.05, F32)
        Ls[:min(n_in, B_TOTAL)] = Ls_in[:B_TOTAL]
    a = np.asarray(a, F32).reshape(-1)
    b = np.asarray(b, F32).reshape(-1)

    host = _host_build(a, b, logcoef)

    L_crit = F32(host["L_crit"])
    valid = Ls < L_crit
    L_eff = np.where(valid, Ls, F32(0.5) * L_crit).astype(F32)
    Lg, zg = host["Lgrid"], host["zgrid"]
    if np.all(np.diff(Lg) > 0):
        init = np.interp(L_eff, Lg, zg).astype(F32)
    else:
        init = np.clip(L_eff / F32(host["L_max"]) * F32(host["zs_max"]),
                       1e-4, 0.9995).astype(F32)

    key = ("graph", host["BH"].tobytes(), host["BJ"].tobytes(),
           F32(host["zcap"]).tobytes(), F32(host["coef"]).tobytes(),
           host["BSGW"].tobytes())
    kh = hash(key)
    if kh not in _CACHE:
        _CACHE[kh] = _build_graph(host)
    nc = _CACHE[kh]

    import ml_dtypes
    bf16 = ml_dtypes.bfloat16
    consts = dict(
        bh=host["BH"], bj=host["BJ"], bsgw=host["BSGW"], bg=host["BG"],
        bfv=host["BFV"], bgv=host["BGV"].astype(bf16),
        phiz2=host["PHI_Z2"], phifd=host["PHI_FD"].astype(bf16),
        phib=host["PHI_B"].astype(bf16),
        phid2=host["PHI_D2"].astype(bf16), phiz4=host["PHI_Z4"].astype(bf16),
        u4c2=host["U4C2"].reshape(1, M),
        iota=np.arange(KZ, dtype=F32).reshape(1, KZ),
        ident=np.eye(P, dtype=F32),
    )

    in_maps = []
    for i in range(N_CORES):
        sl = slice(i * B_CORE, (i + 1) * B_CORE)
        in_maps.append(dict(
            lt=np.ascontiguousarray(L_eff[sl].reshape(NT, P).T),
            init=np.ascontiguousarray(init[sl].reshape(NT, P).T),
            **consts,
        ))

    res = run_bass_kernel_spmd(nc, in_maps, list(range(N_CORES)))
    globals()["_LAST_RESULTS"] = res

    V = np.empty(B_TOTAL, F32)
    for i in range(N_CORES):
        V[i * B_CORE:(i + 1) * B_CORE] = res.results[i]["out"].T.ravel()

    out = np.where(valid, V, np.zeros_like(V)).astype(F32)
    if n_in != B_TOTAL:
        full = np.zeros(n_in, F32)
        full[:min(n_in, B_TOTAL)] = out[:min(n_in, B_TOTAL)]
        return full
    return out



# revision 11
# speedup vs baseline: 3.9213x; 3.9213x over previous
"""Trainium2 Bass kernel for nn_AdSBHNet (holographic Wilson-loop potential).

Strategy (pure data parallel, 8 NeuronCores, 512 batch rows/core as 4x128):
  * Host (numpy, parameter-only work): polynomial coefficients of f/df/B,
    power series for sqrt(g), z*g'/g, g; scalar bisection prelims
    (zs_max, L_max, L_crit) and a 257-point L->zs init grid. All of that
    depends only on the tiny parameters a, b, logcoef.
  * Quadrature: 64-node Gauss-Legendre on y in [0,1] for the L/dL/V
    connected integrals (the integrands are smooth; the apparent sqrt
    singularities cancel analytically, and h-1 is evaluated through a
    cancellation-free basis).  The disconnected V integral must match the
    reference's trapezoid treatment of the 1/z2^2 endpoint spike, so its
    64 nodes are 31-node GL on the smooth bulk [0, 0.968] plus the
    reference's exact last 33 trapezoid nodes/weights.
  * Device per core: one fp32r matmul per tile per phase (all quadrature
    quantities concatenated into a single PSUM bank: moving operand is
    >=256 wide so fp32r runs at 1 cycle/row), stationaries built from
    exp(k*ln zs) via one IOTA matmul + one wide exp, elementwise work done
    as 4-tile-wide strided ops, and per-tile quadrature sums via grouped
    tensor_reduce (axis=X on a [P,4,64] view).  One Newton step for
    L(zs)=L, then V(zs); T1-T2 is accumulated node-wise as
    sum(SQ*(sqrt(h/m)-1)) to avoid catastrophic cancellation at small zs.
Host gathers the 8 per-core [128,4] outputs and applies the validity mask.
"""
import numpy as np
from math import comb

import concourse.bass as bass
import concourse.tile as tile
from concourse import bacc, mybir
from concourse.bass_utils import run_bass_kernel_spmd
from concourse.hw_specs import get_activation_tables
import bass_rust as _bass_rust


class _PinnedActBacc(bacc.Bacc):
    """Bacc that restricts the activation-table chooser to the single
    natural_log_exp_and_others set (covers Ln/Exp/Square/Copy/Identity)
    so Ln<->Exp transitions never reload tables (~2.7us per reload)."""

    _ACT_SET = "natural_log_exp_and_others"

    def insert_act_table_loads(self):
        has_activation = any(
            isinstance(i, mybir.InstActivation)
            for b in self.main_func.blocks
            for i in b.instructions
        )
        if not has_activation:
            return
        tables = []
        for name, funcs in get_activation_tables(self.m.arch).items():
            tables.append((name, funcs if name == self._ACT_SET else set()))
        _bass_rust.insert_act_table_loads(self, tables)


F32 = np.float32
F64 = np.float64
PI = float(np.pi)
EPS = 1e-12
B_TOTAL = 4096
N_CORES = 8
B_CORE = B_TOTAL // N_CORES      # 512
NT = 4                           # row tiles per core
P = 128                          # partitions
M = 1000                         # reference quadrature points (host only)
N = 64                           # device quadrature nodes per integral
KZ = 64                          # zs-power series order
KS = 68                          # stationary rows: zs^0..63, +4 data rows
DT = mybir.dt.float32
DTR = mybir.dt.float32r

_CACHE = {}


# ----------------------------------------------------------------------------
# Host-side math (parameter-only, O(M) work) -- identical to the reference
# ----------------------------------------------------------------------------

def _ygrid():
    return np.linspace(0.001, 0.999, M, dtype=F32).astype(F64)


def _trapz_weights():
    y = _ygrid()
    y0 = y[0]
    h = (y[-1] - y[0]) / (M - 1)
    w = np.full(M, h, F64)
    w[0] = 0.5 * h + y0 + 0.5 * y0 * y0 / h
    w[1] = h - 0.5 * y0 * y0 / h
    w[-1] = 0.5 * h + 0.5 * (1.0 - y[-1])
    return w


def _y2grid():
    return np.linspace(0.001, 1.0, M, dtype=F32).astype(F64)


def _trapz2_weights():
    y2 = _y2grid()
    h2 = (y2[-1] - y2[0]) / (M - 1)
    w2 = np.full(M, h2, F64)
    w2[0] = 0.5 * h2 + 0.5 * y2[0]
    w2[-1] = 0.5 * h2
    return w2, 0.5 * y2[0]


def _f_coeffs(a):
    _a = np.concatenate([np.ones(1, F64), np.asarray(a, F64)])
    A = np.zeros(5, F64)
    q = 0.0
    for i in range(3):
        for j in range(3):
            cc = _a[i] * _a[j]
            if i + j == 4:
                q += -4.0 * cc
            else:
                A[4] += 4.0 * cc / (i + j - 4)
                A[i + j] -= 4.0 * cc / (i + j - 4)
    return A, q


def _df_coeffs(a):
    _a = np.concatenate([np.ones(1, F64), np.asarray(a, F64)])
    A, q = _f_coeffs(a)
    D = 4.0 * A.copy()
    for i in range(3):
        for j in range(3):
            D[i + j] -= 4.0 * _a[i] * _a[j]
    return D, 4.0 * q


def _b_coeffs(a, b):
    last = float(np.asarray(a, F64).sum()) - float(np.asarray(b, F64).sum())
    return np.array([1.0, float(b[0]), float(b[1]), last], F64)


def _series_inv_poly(c, K):
    e = np.zeros(K)
    e[0] = 1.0 / c[0]
    for k in range(1, K):
        s = 0.0
        for j in range(1, min(len(c), k + 1)):
            s += c[j] * e[k - j]
        e[k] = -s / c[0]
    return e


def _conv_trunc(a, b, K):
    return np.convolve(a, b)[:K]


def _build_series(c):
    n_terms = KZ // 4 + 1
    s = np.zeros(n_terms)
    s[0] = 1.0
    for n in range(1, n_terms):
        s[n] = s[n - 1] * (2 * n - 1) / (2 * n)
    rsq = np.zeros(KZ)
    rsq[::4] = s[: len(rsq[::4])]
    sg = _conv_trunc(c, rsq, KZ)                       # B(z)(1-z^4)^-1/2
    zBp = np.array([0.0, c[1], 2 * c[2], 3 * c[3]])
    g1 = 2.0 * _conv_trunc(zBp, _series_inv_poly(c, KZ), KZ)
    g2 = np.zeros(KZ)
    g2[4::4] = 4.0
    gg = g1 + g2                                       # z g'/g
    inv1mz4 = np.zeros(KZ)
    inv1mz4[::4] = 1.0
    gser = _conv_trunc(_conv_trunc(c, c, KZ), inv1mz4, KZ)  # B^2/(1-z^4)
    return sg, gg, gser


class _HostModel:
    """float32 replica of the reference for the scalar bisection prelims."""

    def __init__(self, a, b):
        self.A, self.q = _f_coeffs(a)
        self.D, self.dq = _df_coeffs(a)
        self.c = _b_coeffs(a, b)
        self.y = _ygrid().astype(F32)
        self.u = ((1 - self.y) * (1 + self.y)).astype(F32)
        self.w = _trapz_weights().astype(F32)
        self.y2 = _y2grid().astype(F32)
        w2, c2 = _trapz2_weights()
        self.w2 = w2.astype(F32)
        self.c2 = F32(c2)

    def _f(self, z, lnz):
        A, q = self.A, self.q
        return (A[4] * z**4 + A[3] * z**3 + A[2] * z**2 + A[1] * z + A[0]
                + q * z**4 * lnz).astype(F32)

    def _df(self, z, lnz):
        D, dq = self.D, self.dq
        return (D[0] / z + D[1] + D[2] * z + D[3] * z**2 + D[4] * z**3
                + dq * z**3 * lnz).astype(F32)

    def L_dL(self, zs):
        zs = np.asarray(zs, F32).reshape(-1)[:, None]
        u, y, w = self.u[None, :], self.y[None, :], self.w
        z = (zs * u).astype(F32)
        lnz = np.log(z)
        lnzs = np.log(zs)
        fs = self._f(zs, lnzs)
        dfs = self._df(zs, lnzs)
        rfs = (1.0 / fs).astype(F32)
        f = self._f(z, lnz)
        c = self.c
        Bv = (c[0] + c[1] * z + c[2] * z**2 + c[3] * z**3).astype(F32)
        Bp = (c[1] + 2 * c[2] * z + 3 * c[3] * z**2).astype(F32)
        D_ = (1 - z**4).astype(F32)
        sqrtg = (Bv / np.sqrt(D_)).astype(F32)
        h = (f * rfs / u**4).astype(F32)
        m = np.maximum(h - 1, F32(EPS))
        R = (1.0 / np.sqrt(m)).astype(F32)
        TL = ((sqrtg * R * y * w).sum(-1, dtype=F64)).astype(F32)
        L = (4.0 * zs[:, 0] * TL / PI).astype(F32)
        G = (2 * z * Bp / Bv + 4 * z**4 / D_).astype(F32)
        sA = (zs * dfs * rfs + 2).astype(F32)
        J = (zs**4 / z**3 * self._df(z, lnz) * rfs).astype(F32)
        v = (h * (sA + G) - J - 2 - G).astype(F32)
        IdL = (v * 2 * y * sqrtg * R / m).astype(F32)
        dL = ((IdL * w).sum(-1, dtype=F64) / PI).astype(F32)
        return L, dL

    def V(self, zs, coef):
        zs = np.asarray(zs, F32).reshape(-1)[:, None]
        u, y, w = self.u[None, :], self.y[None, :], self.w
        z = (zs * u).astype(F32)
        lnz = np.log(z)
        lnzs = np.log(zs)
        fs = self._f(zs, lnzs)
        f = self._f(z, lnz)
        c = self.c
        Bv = (c[0] + c[1] * z + c[2] * z**2 + c[3] * z**3).astype(F32)
        g = (Bv * Bv / (1 - z**4)).astype(F32)
        fg = np.maximum(f * g, F32(EPS))
        arg = np.maximum(1 - u**4 * fs / f, F32(EPS))
        integ = (np.sqrt(fg) / u**2 * (1 / np.sqrt(arg) - 1) * y).astype(F32)
        Vc = (coef * PI * 4.0 * (integ * w).sum(-1, dtype=F64) / zs[:, 0]).astype(F32)
        y2, w2 = self.y2[None, :], self.w2
        z2 = (1 - (1 - zs) * y2).astype(F32)
        f2 = self._f(z2, np.log(z2))
        B2 = (c[0] + c[1] * z2 + c[2] * z2**2 + c[3] * z2**3).astype(F32)
        g2 = (B2 * B2 / (1 - z2**4)).astype(F32)
        fg2 = np.maximum(f2 * g2, F32(EPS))
        integ2 = (np.sqrt(fg2) / z2**2).astype(F32)
        Vd = (coef * PI * 2.0 * (1 - zs[:, 0])
              * ((integ2 * w2).sum(-1, dtype=F64) + self.c2)).astype(F32)
        return (Vc - Vd).astype(F32)

    def bisect(self, fun, lo, hi, iters=30):
        lo, hi = F32(lo), F32(hi)
        for _ in range(iters):
            mid = F32(0.5) * (lo + hi)
            if fun(mid) < 0:
                hi = mid
            else:
                lo = mid
        return F32(0.5) * (lo + hi)

    def prelims(self, coef):
        zs_max = self.bisect(lambda mm: self.L_dL(mm)[1][0], 0.001, 0.999)
        L_max = self.L_dL(zs_max)[0][0]
        zs_crit = self.bisect(lambda mm: -self.V(mm, coef)[0], 0.001, zs_max)
        L_crit = self.L_dL(zs_crit)[0][0]
        return zs_max, L_max, zs_crit, L_crit


def _gl_nodes(n):
    x, w = np.polynomial.legendre.leggauss(n)
    return 0.5 * (x + 1.0), 0.5 * w


def _host_build(a, b, logcoef):
    """All parameter-derived constants + the concatenated basis matrix."""
    A, q = _f_coeffs(a)
    Dc, dq = _df_coeffs(a)
    c = _b_coeffs(a, b)
    sg, gg, gser = _build_series(c)
    coef = float(np.exp(F32(np.asarray(logcoef).reshape(-1)[0]
                            if np.ndim(logcoef) else logcoef)))

    mdl = _HostModel(a, b)
    zs_max, L_max, zs_crit, L_crit = mdl.prelims(coef)
    zcap = float(min(0.9995, float(zs_max) * 0.97))
    zgrid = np.linspace(1e-4, zcap, 257).astype(F32)
    Lgrid = mdl.L_dL(zgrid)[0]

    # ---- shared GL grid for L/dL/V-connected ----
    y, w = _gl_nodes(N)
    u = 1 - y * y
    lnu = np.log(u)
    ru = 1.0 / u
    yw = y * w
    ks = np.arange(KZ)[:, None]
    zero = np.zeros(N)
    one = np.ones(N)

    def blk(rows):
        """rows: dict row_index -> vector; -> [KS, N] block."""
        B = np.zeros((KS, N))
        for k, v in rows.items():
            B[k] = v
        return B

    # Newton blocks.  Stationary rows: 0..63 = zs^k, 64 = ln zs (dummy),
    # 65 = sA, 66 = zs^4 ln zs, 67 = -zs*dfs.
    MB = blk({0: A[0] * (ru**4 - 1), 1: A[1] * (ru**3 - 1),
              2: A[2] * (ru**2 - 1), 3: A[3] * (ru - 1), 4: q * lnu})
    SGW = blk({k: sg[k] * u**k * yw for k in range(KZ)})
    GSA = blk({**{k: gg[k] * u**k for k in range(KZ)}, 65: one})
    JF = blk({0: Dc[0] * ru**4, 1: Dc[1] * ru**3, 2: Dc[2] * ru**2,
              3: Dc[3] * ru, 4: Dc[4] + dq * lnu, 66: dq * one, 67: one})
    BN = np.concatenate([MB, SGW, GSA, JF], axis=1)        # [KS, 4N]

    # V blocks.  Stationary rows: 0..63 = zs^k, 64 = ln zs (dummy),
    # 65 = zeros, 66 = zs^4 ln zs, 67 = zeros.
    cw = yw / u**2
    FV = blk({0: A[0] * cw**2, 1: A[1] * u * cw**2, 2: A[2] * u**2 * cw**2,
              3: A[3] * u**3 * cw**2, 4: (A[4] + q * lnu) * u**4 * cw**2,
              66: q * u**4 * cw**2})
    GV = blk({k: gser[k] * u**k for k in range(KZ)})

    # hybrid disconnected grid: GL bulk + exact reference trapz tail
    h2 = 0.999 / (M - 1)
    n_tail = N // 2                                        # 32 intervals
    n_gl = N - n_tail - 1                                  # 31 GL nodes
    y2_B = 0.001 + (M - 1 - n_tail) * h2
    yg, wg = _gl_nodes(n_gl)
    y2 = np.concatenate([yg * y2_B,
                         0.001 + np.arange(M - 1 - n_tail, M) * h2])
    w2 = np.concatenate([wg * y2_B, np.full(n_tail + 1, h2)])
    w2[n_gl] = 0.5 * h2
    w2[-1] = 0.5 * h2
    alpha, beta = 1 - y2, y2
    w2s = w2 * w2

    def phi(coefs, extra, mmax):
        rows = {}
        for mdeg in range(mmax):
            r = np.zeros(N)
            for k in range(mdeg, len(coefs)):
                if coefs[k] != 0:
                    r += coefs[k] * comb(k, mdeg) * alpha**(k - mdeg) * beta**mdeg
            rows[mdeg] = r * extra
        return blk(rows)

    Z2B = phi([0, 1], one, 2)
    FDW = phi(list(A), w2s, 5)
    B2D = phi(list(c), one, 4)
    D2B = phi([1, 0, 0, 0, -1], one, 5)
    Z4W = phi([0, 0, 0, 0, 1], w2s, 5)
    BV = np.concatenate([FV, GV, MB, Z2B, FDW, B2D, D2B, Z4W], axis=1)

    # prepend-at-0 correction: reference used value 1 at y2=0; the GL bulk
    # integrates the true limit F(0) = sqrt(-f'(1) B(1)^2 / 4)
    fp1 = A[1] + 2 * A[2] + 3 * A[3] + 4 * A[4] + q
    F0 = float(np.sqrt(max(-fp1, 0.0) * float(np.sum(c)) ** 2 / 4.0))
    c2 = 0.5 * 0.001 * (1.0 - F0)

    iota = np.zeros((KS, KZ))
    iota[0] = np.arange(KZ)
    BASIS = np.concatenate([BN, BV, iota], axis=1).astype(F32)  # [KS, 832]

    return dict(
        A=A, q=q, Dc=Dc, dq=dq, c=c, coef=coef, c2=float(c2),
        zs_max=float(zs_max), L_max=float(L_max), L_crit=float(L_crit),
        zcap=zcap, zgrid=zgrid, Lgrid=Lgrid, BASIS=BASIS,
    )


# ----------------------------------------------------------------------------
# Device graph
# ----------------------------------------------------------------------------

def _build_graph(host):
    A, q, Dc, dq = host["A"], host["q"], host["Dc"], host["dq"]
    coef, c2 = host["coef"], host["c2"]
    zcap = host["zcap"]
    f32 = lambda x: float(F32(x))
    alu = mybir.AluOpType
    act = mybir.ActivationFunctionType
    AX = mybir.AxisListType

    nc = _PinnedActBacc("TRN2", target_bir_lowering=False, debug=False,
                        num_devices=N_CORES)

    ltinit_ext = nc.declare_dram_parameter("ltinit", [P, 2 * NT], DT,
                                           isOutput=False)
    basis_ext = nc.declare_dram_parameter("basis", [KS, 13 * N], DTR,
                                          isOutput=False)
    ident_ext = nc.declare_dram_parameter("ident", [P, P], DT, isOutput=False)
    out_ext = nc.declare_dram_parameter("out", [P, NT], DT, isOutput=True)

    W = NT * N                       # 256: width of 4-tile packed ops
    OB = 512                         # psum col offset between tiles (1 bank)

    with tile.TileContext(nc) as tc:
        with (
            tc.tile_pool(name="const", bufs=1) as cpool,
            tc.tile_pool(name="small", bufs=3) as smpool,
            tc.tile_pool(name="sc", bufs=1) as scpool,
            tc.tile_pool(name="stat", bufs=2) as stpool,
            tc.tile_pool(name="wide", bufs=2) as wpool,
            tc.tile_pool(name="psum", bufs=2, space="PSUM") as ppool,
        ):
            LTINIT = cpool.tile([P, 2 * NT], DT, tag="c_ltinit")
            nc.sync.dma_start(LTINIT[:], ltinit_ext[:])
            BASIS = cpool.tile([KS, 13 * N], DTR, tag="c_basis")
            nc.sync.dma_start(BASIS[:], basis_ext[:])
            IDENT = cpool.tile([P, P], DT, tag="c_ident")
            nc.sync.dma_start(IDENT[:], ident_ext[:])

            LT = LTINIT[:, 0:NT]
            ZS0 = LTINIT[:, NT:2 * NT]
            BN_ = BASIS[:, 0:4 * N]
            BV_ = BASIS[:, 4 * N:12 * N]
            IOTA = BASIS[0:1, 12 * N:12 * N + KZ]

            def small(tag):
                return smpool.tile([P, NT], DT, tag=tag, name=tag)

            def q_slice(t, nq, qi):
                return t[:].rearrange("p (t q) -> p t q", q=nq)[:, :, qi]

            def tiles_view(t, width=N):
                """[P, NT*width] contiguous -> [P, NT, width]"""
                return t[:].rearrange("p (t m) -> p t m", m=width)

            def pblk(ps, off, width=N):
                """strided view of psum [P, 2048]: per-tile block at
                col offset `off` within each 512-col bank."""
                v = ps[:].rearrange("p (t m) -> p t m", m=OB)
                return v[:, :, off:off + width]

            def mm_r(out_ap, lhsT_ap, rhs_ap):
                nc.tensor.matmul(out_ap, lhsT_ap, rhs_ap)

            # ============ scalar phase at ZS (shared helper) ============
            def scalar_phase(ZS, newton):
                """returns dict of [P,NT] tiles + SC3 assembly tile."""
                nq = 4
                SC3 = scpool.tile([P, NT * nq], DT, name="sc3",
                                  tag="sc3n" if newton else "sc3v")
                # lnzs slot doubles as the KLN rhs row: always q0 so the
                # matmul moving operand starts at base partition 0
                LNZS = q_slice(SC3, nq, 0)
                nc.scalar.activation(LNZS, ZS, act.Ln)
                ZS2 = small("zs2")
                nc.vector.tensor_mul(ZS2[:], ZS, ZS)
                ZS3 = small("zs3")
                nc.vector.tensor_mul(ZS3[:], ZS2[:], ZS)
                ZS4 = small("zs4")
                nc.vector.tensor_mul(ZS4[:], ZS2[:], ZS2[:])
                LZ4 = q_slice(SC3, nq, 2)
                nc.vector.tensor_mul(LZ4, ZS4[:], LNZS)
                FS = small("fs")
                t1 = small("tmp1")
                nc.vector.tensor_scalar(t1[:], ZS, f32(A[1]), f32(A[0]),
                                        alu.mult, alu.add)
                t2 = small("tmp2")
                nc.vector.scalar_tensor_tensor(t2[:], ZS2[:], f32(A[2]), t1[:],
                                               alu.mult, alu.add)
                nc.vector.scalar_tensor_tensor(t1[:], ZS3[:], f32(A[3]), t2[:],
                                               alu.mult, alu.add)
                nc.vector.scalar_tensor_tensor(t2[:], ZS4[:], f32(A[4]), t1[:],
                                               alu.mult, alu.add)
                nc.vector.scalar_tensor_tensor(FS[:], LZ4, f32(q), t2[:],
                                               alu.mult, alu.add)
                out = dict(SC3=SC3, FS=FS, nq=nq)
                if newton:
                    LZ3 = small("lz3")
                    nc.vector.tensor_mul(LZ3[:], ZS3[:], LNZS)
                    RZS = small("rzs")
                    nc.vector.reciprocal(RZS[:], ZS)
                    DFS = small("dfs")
                    t3 = small("tmp3")
                    nc.vector.tensor_scalar(t3[:], ZS, f32(Dc[2]), f32(Dc[1]),
                                            alu.mult, alu.add)
                    t4 = small("tmp4")
                    nc.vector.scalar_tensor_tensor(t4[:], ZS2[:], f32(Dc[3]),
                                                   t3[:], alu.mult, alu.add)
                    nc.vector.scalar_tensor_tensor(t3[:], ZS3[:], f32(Dc[4]),
                                                   t4[:], alu.mult, alu.add)
                    nc.vector.scalar_tensor_tensor(t4[:], RZS[:], f32(Dc[0]),
                                                   t3[:], alu.mult, alu.add)
                    nc.vector.scalar_tensor_tensor(DFS[:], LZ3[:], f32(dq),
                                                   t4[:], alu.mult, alu.add)
                    RFS = small("rfs")
                    nc.vector.reciprocal(RFS[:], FS[:])
                    LNFS = small("lnfs")
                    nc.scalar.activation(LNFS[:], FS[:], act.Ln)
                    SQFS = small("sqfs")
                    nc.scalar.activation(SQFS[:], LNFS[:], act.Exp, scale=0.5)
                    T0 = small("t0")
                    nc.vector.tensor_mul(T0[:], ZS, DFS[:])
                    nc.vector.tensor_scalar(q_slice(SC3, nq, 3), T0[:], -1.0,
                                            None, alu.mult)          # -zs*dfs
                    TR = small("tr")
                    nc.vector.tensor_mul(TR[:], T0[:], RFS[:])
                    nc.vector.tensor_scalar(q_slice(SC3, nq, 1), TR[:], 2.0,
                                            None, alu.add)           # sA
                    out["SQFS"] = SQFS
                else:
                    nc.vector.memset(q_slice(SC3, nq, 1), 0.0)
                    nc.vector.memset(q_slice(SC3, nq, 3), 0.0)
                return out

            # ============ stationary build (shared helper) ============
            def build_stationary(SC3, nq, tag):
                # newton rows: {0: lnzs, 1: sA, 2: zs^4 lnzs, 3: -zs dfs}
                # V rows:      {0: lnzs, 1: 0, 2: zs^4 lnzs, 3: 0}
                TPS = ppool.tile([P, NT * OB], DT, tag="ps", name="tps")
                for t in range(NT):
                    nc.tensor.transpose(TPS[0:nq, t * OB:t * OB + P],
                                        SC3[:, t * nq:(t + 1) * nq],
                                        IDENT[:])
                S3 = stpool.tile([nq, NT * P], DTR, tag=f"s3_{tag}", name="s3")
                nc.vector.tensor_scalar(
                    S3[:], pblk(TPS, 0, P)[0:nq], 1.0, None, alu.mult)
                KLNP = ppool.tile([P, NT * OB], DT, tag="ps", name="klnp")
                for t in range(NT):
                    nc.tensor.matmul(
                        KLNP[0:KZ, t * OB:t * OB + P], IOTA,
                        S3[0:1, t * P:(t + 1) * P])
                S2V = stpool.tile([KS, NT * P], DTR, tag=f"s2v_{tag}", name="s2v")
                nc.scalar.activation(S2V[0:KZ, :], pblk(KLNP, 0, P)[0:KZ],
                                     act.Exp)
                nc.gpsimd.tensor_copy(S2V[KZ:KZ + 4, :], S3[0:4, :])
                return S2V

            # ===================== Newton step =====================
            sc = scalar_phase(ZS0, newton=True)
            S2V = build_stationary(sc["SC3"], sc["nq"], "n")
            NPS = ppool.tile([P, NT * OB], DT, tag="ps")
            for t in range(NT):
                mm_r(NPS[:, t * OB:t * OB + 4 * N],
                     S2V[:, t * P:(t + 1) * P], BN_)

            LNM = wpool.tile([P, W], DT, tag="w_lnm")
            nc.scalar.activation(LNM[:], pblk(NPS, 0), act.Ln)
            R = wpool.tile([P, W], DT, tag="w_r")
            nc.scalar.activation(R[:], LNM[:], act.Exp, scale=-0.5)
            RM = wpool.tile([P, W], DT, tag="w_rm")
            nc.gpsimd.tensor_mul(RM[:], R[:], R[:])
            SW = wpool.tile([P, W], DT, tag="w_sw")
            nc.vector.tensor_mul(SW[:], pblk(NPS, N), R[:])
            P1 = wpool.tile([P, W], DT, tag="w_p1")
            nc.vector.tensor_mul(P1[:], pblk(NPS, 2 * N), SW[:])
            RSW = wpool.tile([P, W], DT, tag="w_rsw")
            nc.gpsimd.tensor_mul(RSW[:], RM[:], SW[:])
            P2 = wpool.tile([P, W], DT, tag="w_p2")
            nc.vector.tensor_mul(P2[:], pblk(NPS, 3 * N), RSW[:])
            TLp = small("tlp")
            nc.vector.tensor_reduce(TLp[:], tiles_view(SW), AX.X, alu.add)
            TD1p = small("td1p")
            nc.vector.tensor_reduce(TD1p[:], tiles_view(P1), AX.X, alu.add)
            TD2p = small("td2p")
            nc.vector.tensor_reduce(TD2p[:], tiles_view(P2), AX.X, alu.add)

            TDp = small("tdp")
            nc.vector.tensor_sub(TDp[:], TD1p[:], TD2p[:])
            SQFS = sc["SQFS"]
            TL = small("tl")
            nc.vector.tensor_mul(TL[:], TLp[:], SQFS[:])
            TD = small("td")
            nc.vector.tensor_mul(TD[:], TDp[:], SQFS[:])
            T1f = small("t1f")
            nc.vector.tensor_mul(T1f[:], ZS0, TL[:])
            RT = small("rt")
            nc.vector.reciprocal(RT[:], TD[:])
            LMF = small("lmf")
            nc.vector.scalar_tensor_tensor(LMF[:], T1f[:], f32(4.0 / PI),
                                           LT, alu.mult, alu.subtract)
            DEL = small("del")
            nc.vector.scalar_tensor_tensor(DEL[:], LMF[:], f32(PI / 2),
                                           RT[:], alu.mult, alu.mult)
            ZSn = small("zsn")
            nc.vector.tensor_sub(ZSn[:], ZS0, DEL[:])
            ZS1 = smpool.tile([P, NT], DT, tag="zs1")
            nc.vector.tensor_scalar(ZS1[:], ZSn[:], 1e-4, zcap,
                                    alu.max, alu.min)

            # ===================== V phase =====================
            scv = scalar_phase(ZS1[:], newton=False)
            FSV = scv["FS"]
            S2W = build_stationary(scv["SC3"], scv["nq"], "v")
            VPS = ppool.tile([P, NT * OB], DT, tag="ps")
            for t in range(NT):
                mm_r(VPS[:, t * OB:t * OB + 8 * N],
                     S2W[:, t * P:(t + 1) * P], BV_)

            # connected: blocks FV@0, GV@N, M2@2N, Z2@3N, FDW@4N, B2D@5N,
            # D2@6N, Z4W@7N
            GVS = wpool.tile([P, W], DT, tag="w_gvs")
            nc.scalar.copy(GVS[:], pblk(VPS, N))
            FG = wpool.tile([P, W], DT, tag="w_fg")
            nc.vector.tensor_mul(FG[:], pblk(VPS, 0), GVS[:])
            LFG = wpool.tile([P, W], DT, tag="w_lfg")
            nc.scalar.activation(LFG[:], FG[:], act.Ln)
            SQ = wpool.tile([P, W], DT, tag="w_sq")
            nc.scalar.activation(SQ[:], LFG[:], act.Exp, scale=0.5)
            RMF = wpool.tile([P, W], DT, tag="w_rmf")
            nc.vector.reciprocal(RMF[:], pblk(VPS, 2 * N))
            X = wpool.tile([P, W], DT, tag="w_x")
            for t in range(NT):
                nc.vector.tensor_scalar(X[:, t * N:(t + 1) * N],
                                        RMF[:, t * N:(t + 1) * N],
                                        FSV[:, t:t + 1], None, alu.mult)
            L1X = wpool.tile([P, W], DT, tag="w_l1x")
            nc.scalar.activation(L1X[:], X[:], act.Ln, bias=1.0, scale=1.0)
            SHM = wpool.tile([P, W], DT, tag="w_shm")
            nc.scalar.activation(SHM[:], L1X[:], act.Exp, scale=0.5)
            D1 = wpool.tile([P, W], DT, tag="w_d1")
            nc.vector.tensor_scalar(D1[:], SHM[:], 1.0, None, alu.subtract)
            PQ = wpool.tile([P, W], DT, tag="w_pq")
            nc.vector.tensor_mul(PQ[:], SQ[:], D1[:])
            T12 = small("t12")
            nc.vector.tensor_reduce(T12[:], tiles_view(PQ), AX.X, alu.add)

            # disconnected
            LZ2 = wpool.tile([P, W], DT, tag="w_lz2")
            nc.scalar.activation(LZ2[:], pblk(VPS, 3 * N), act.Ln)
            ID2 = wpool.tile([P, W], DT, tag="w_id2")
            nc.vector.reciprocal(ID2[:], pblk(VPS, 6 * N))
            B2S = wpool.tile([P, W], DT, tag="w_b2s")
            nc.scalar.activation(B2S[:], pblk(VPS, 5 * N), act.Square)
            TLm = wpool.tile([P, W], DT, tag="w_tlm")
            nc.vector.tensor_mul(TLm[:], pblk(VPS, 7 * N), LZ2[:])
            FD2 = wpool.tile([P, W], DT, tag="w_fd2")
            nc.vector.scalar_tensor_tensor(FD2[:], TLm[:], f32(q),
                                           pblk(VPS, 4 * N), alu.mult, alu.add)
            FG2a = wpool.tile([P, W], DT, tag="w_fg2a")
            nc.gpsimd.tensor_mul(FG2a[:], FD2[:], ID2[:])
            FG2 = wpool.tile([P, W], DT, tag="w_fg2")
            nc.gpsimd.tensor_mul(FG2[:], FG2a[:], B2S[:])
            LF2 = wpool.tile([P, W], DT, tag="w_lf2")
            nc.scalar.activation(LF2[:], FG2[:], act.Ln)
            LCOD = wpool.tile([P, W], DT, tag="w_lcod")
            nc.vector.scalar_tensor_tensor(LCOD[:], LZ2[:], -4.0, LF2[:],
                                           alu.mult, alu.add)
            EXPD = wpool.tile([P, W], DT, tag="w_expd")
            nc.scalar.activation(EXPD[:], LCOD[:], act.Exp, scale=0.5)
            TDd = small("tdd")
            nc.vector.tensor_reduce(TDd[:], tiles_view(EXPD), AX.X, alu.add)

            # ---- finalize ----
            RZSV = small("rzsv")
            nc.vector.reciprocal(RZSV[:], ZS1[:])
            VC1 = small("vc1")
            nc.vector.tensor_mul(VC1[:], T12[:], RZSV[:])
            O1 = small("o1")
            nc.vector.tensor_scalar(O1[:], VC1[:], f32(4.0 * PI * coef),
                                    None, alu.mult)
            TVD = small("tvd")
            nc.vector.tensor_scalar(TVD[:], TDd[:], f32(c2), None, alu.add)
            OMZ = small("omz")
            nc.vector.tensor_scalar(OMZ[:], ZS1[:], -1.0, 1.0,
                                    alu.mult, alu.add)
            VD1 = small("vd1")
            nc.vector.tensor_mul(VD1[:], TVD[:], OMZ[:])
            OUT = small("outt")
            nc.vector.scalar_tensor_tensor(OUT[:], VD1[:],
                                           f32(-2.0 * PI * coef), O1[:],
                                           alu.mult, alu.add)
            nc.sync.dma_start(out_ext[:], OUT[:])

    nc.compile()
    return nc


# ----------------------------------------------------------------------------
# kernel entry point
# ----------------------------------------------------------------------------

def kernel(Ls, a, b, logcoef):
    Ls_in = np.asarray(Ls, F32).reshape(-1)
    n_in = Ls_in.size
    if n_in == B_TOTAL:
        Ls = Ls_in
    else:
        Ls = np.full(B_TOTAL, 0.05, F32)
        Ls[:min(n_in, B_TOTAL)] = Ls_in[:B_TOTAL]
    a = np.asarray(a, F32).reshape(-1)
    b = np.asarray(b, F32).reshape(-1)

    host = _host_build(a, b, logcoef)

    L_crit = F32(host["L_crit"])
    valid = Ls < L_crit
    L_eff = np.where(valid, Ls, F32(0.5) * L_crit).astype(F32)
    Lg, zg = host["Lgrid"], host["zgrid"]
    if np.all(np.diff(Lg) > 0):
        init = np.interp(L_eff, Lg, zg).astype(F32)
    else:
        init = np.clip(L_eff / F32(host["L_max"]) * F32(host["zs_max"]),
                       1e-4, 0.9995).astype(F32)

    key = ("graph2", host["BASIS"].tobytes(), F32(host["zcap"]).tobytes(),
           F32(host["coef"]).tobytes(), F32(host["c2"]).tobytes())
    kh = hash(key)
    if kh not in _CACHE:
        _CACHE[kh] = _build_graph(host)
    nc = _CACHE[kh]

    consts = dict(
        basis=host["BASIS"],
        ident=np.eye(P, dtype=F32),
    )

    in_maps = []
    for i in range(N_CORES):
        sl = slice(i * B_CORE, (i + 1) * B_CORE)
        lt = np.ascontiguousarray(L_eff[sl].reshape(NT, P).T)
        zi = np.ascontiguousarray(init[sl].reshape(NT, P).T)
        in_maps.append(dict(
            ltinit=np.concatenate([lt, zi], axis=1),
            **consts,
        ))

    res = run_bass_kernel_spmd(nc, in_maps, list(range(N_CORES)))
    globals()["_LAST_RESULTS"] = res

    V = np.empty(B_TOTAL, F32)
    for i in range(N_CORES):
        V[i * B_CORE:(i + 1) * B_CORE] = res.results[i]["out"].T.ravel()

    out = np.where(valid, V, np.zeros_like(V)).astype(F32)
    if n_in != B_TOTAL:
        full = np.zeros(n_in, F32)
        full[:min(n_in, B_TOTAL)] = out[:min(n_in, B_TOTAL)]
        return full
    return out
